# revision 1
# baseline (speedup 1.0000x reference)
"""Tensor-parallel causal self-attention on 8 TRN2 NeuronCores.

Sharding: head-parallel. Core r owns heads {2r, 2r+1} (256 of 2048 qkv
features). qkv weight column-sharded, proj weight row-sharded; each core
returns a partial projection output [2048, 4096] (feature-major), host sums
across cores and transposes back to (B, T, C).

On-core dataflow (all feature-major: features on partitions, tokens free):
  xT [2048c, 4096t] (replicated)
  qkv proj  : psum[f,t] += W[c,f]^T x[c,t]  (float32r, full PE rate)
  RoPE      : pair-swap via matmul with a 128x128 swap matrix + 3 DVE ops
  scores    : S^T[j,i] = k^T[d,j]^T @ q^T[d,i]  (bf16)
  softmax   : P = exp(S*scale) (no max-sub; logits are O(1)), causal via
              multiplicative bf16 masks on diagonal blocks, denominator via
              ones-matmul accumulated in psum
  PV        : O^T[d,i] += v[j,d]^T @ P^T[j,i]  (bf16), then divide by denom
  proj      : out[u,t] += wp[dl,u]^T O^T[dl,t]  (bf16)
"""

import sys

if '/opt/trn_rl_repo' not in sys.path:
    sys.path.insert(0, '/opt/trn_rl_repo')

import numpy as np
import ml_dtypes

B, T, C = 2, 2048, 2048
H, HD = 16, 128
NCORES = 8
HLOC = H // NCORES          # 2 heads per core
FLOC = HLOC * HD            # 256 features per core
BT = B * T                  # 4096 tokens
CT = C // 128               # 16 contraction tiles
TS = 512                    # token slice
NTS = T // TS               # 4 t-slices per batch
NJT = T // 128              # 16 key tiles per batch
SCALE = 1.0 / float(np.sqrt(HD))

_cache = {}
OPTS = {}


def _build_nc(loop_n=None, phases="ABC"):
    """Build the SPMD kernel. loop_n wraps the whole computation in an
    on-device For_i loop (benchmarking only — amortizes dispatch overhead).
    phases: "A" qkv only, "AR" +rope/vt, "AB" +attention, "ABC" full."""
    import contextlib

    import concourse.bacc as bacc
    import concourse.mybir as mybir
    from concourse.tile import TileContext

    f32 = mybir.dt.float32
    f32r = mybir.dt.float32r
    bf16 = mybir.dt.bfloat16

    nc = bacc.Bacc("TRN2", target_bir_lowering=False, debug=False,
                   enable_partition_id=False)

    xt_d = nc.dram_tensor("xt", [C, BT], f32r, kind="ExternalInput")
    wq_d = nc.dram_tensor("wq", [C, FLOC], f32r, kind="ExternalInput")
    wk_d = nc.dram_tensor("wk", [C, FLOC], f32r, kind="ExternalInput")
    wv_d = nc.dram_tensor("wv", [C, FLOC], f32r, kind="ExternalInput")
    wp_d = nc.dram_tensor("wp", [FLOC, C], bf16, kind="ExternalInput")
    cos_d = nc.dram_tensor("cos2", [128, T], bf16, kind="ExternalInput")
    sin_d = nc.dram_tensor("sin2", [128, T], bf16, kind="ExternalInput")
    psw_d = nc.dram_tensor("psw", [128, 128], bf16, kind="ExternalInput")
    idn_d = nc.dram_tensor("idn", [128, 128], bf16, kind="ExternalInput")
    one_d = nc.dram_tensor("ones", [128, 1], bf16, kind="ExternalInput")
    msk_d = nc.dram_tensor("masks", [4, 128, TS], bf16, kind="ExternalInput")
    out_d = nc.dram_tensor("out", [C, BT], bf16, kind="ExternalOutput")

    with TileContext(nc) as tc:
        with (
            tc.tile_pool(name="cpool", bufs=1) as cpool,
            tc.tile_pool(name="wpool", bufs=1) as wpool,
            tc.tile_pool(name="xpool", bufs=20) as xpool,
            tc.tile_pool(name="accpool", bufs=1) as accpool,
            tc.tile_pool(name="vpool", bufs=2) as vpool,
            tc.tile_pool(name="tpool", bufs=3) as tpool,
            tc.tile_pool(name="ppool", bufs=4) as ppool,
            tc.tile_pool(name="opool", bufs=3) as opool,
            tc.tile_pool(name="stpool", bufs=4) as stpool,
            tc.tile_pool(name="dpool", bufs=2) as dpool,
            tc.tile_pool(name="ps", bufs=6, space="PSUM") as ps,
        ):
            # ---- constants & weights (resident) ----
            cos_sb = cpool.tile([128, T], bf16, name="cos_sb")
            nc.sync.dma_start(cos_sb, cos_d[:, :])
            sin_sb = cpool.tile([128, T], bf16, name="sin_sb")
            nc.sync.dma_start(sin_sb, sin_d[:, :])
            psw_sb = cpool.tile([128, 128], bf16, name="psw_sb")
            nc.sync.dma_start(psw_sb, psw_d[:, :])
            idn_sb = cpool.tile([128, 128], bf16, name="idn_sb")
            nc.sync.dma_start(idn_sb, idn_d[:, :])
            one_sb = cpool.tile([128, 1], bf16, name="one_sb")
            nc.sync.dma_start(one_sb, one_d[:, :])
            msk_sb = cpool.tile([128, 4, TS], bf16, name="msk_sb")
            nc.sync.dma_start(msk_sb, msk_d.rearrange("q p i -> p q i"))

            w_sb = {}
            for nm, d in (("q", wq_d), ("k", wk_d), ("v", wv_d)):
                w = wpool.tile([128, CT, FLOC], f32r, name=f"w{nm}_sb")
                nc.sync.dma_start(w, d.rearrange("(ct p) f -> p ct f", p=128))
                w_sb[nm] = w
            wp_sb = wpool.tile([128, HLOC, C], bf16, name="wp_sb")
            nc.sync.dma_start(wp_sb, wp_d.rearrange("(hh p) u -> p hh u", p=128))

            loop_cm = (tc.For_i(0, loop_n, 1) if loop_n
                       else contextlib.nullcontext())
            with loop_cm:
              for b in range(B):
                t0 = b * T
                # ---- phase A: qkv projection (float32r) ----
                acc = {}
                for qk in ("q", "k"):
                    for h in range(HLOC):
                        acc[(qk, h)] = accpool.tile(
                            [128, T], bf16, name=f"acc_{qk}{h}_{b}")
                vtok = vpool.tile([128, NJT, FLOC], bf16, tag="vtok",
                                  name=f"vtok_{b}")

                for ts in range(NTS):
                    xts = []
                    for c in range(CT):
                        xt_t = xpool.tile([128, TS], f32r, tag="xt_t",
                                          name="xt_t")
                        nc.sync.dma_start(
                            xt_t,
                            xt_d[c * 128:(c + 1) * 128,
                                 t0 + ts * TS: t0 + (ts + 1) * TS])
                        xts.append(xt_t)
                    for nm in ("q", "k"):
                        for h in range(HLOC):
                            pq = ps.tile([128, TS], f32, tag="mm", bufs=3,
                                         name="ps_qkv")
                            for c in range(CT):
                                nc.tensor.matmul(
                                    pq,
                                    lhsT=w_sb[nm][:, c, h * 128:(h + 1) * 128],
                                    rhs=xts[c],
                                    start=(c == 0), stop=(c == CT - 1))
                            dst = acc[(nm, h)][:, ts * TS:(ts + 1) * TS]
                            nc.scalar.copy(dst, pq)
                    # v directly in token-major: out[t, f] with xT stationary
                    for tt in range(4):
                        pv = ps.tile([128, FLOC], f32, tag="mm", bufs=3,
                                     name="ps_v")
                        for c in range(CT):
                            nc.tensor.matmul(
                                pv,
                                lhsT=xts[c][:, tt * 128:(tt + 1) * 128],
                                rhs=w_sb["v"][:, c, :],
                                start=(c == 0), stop=(c == CT - 1))
                        nc.scalar.copy(vtok[:, ts * 4 + tt, :], pv)

                if phases == "A":
                    nc.gpsimd.dma_start(out_d[0:128, t0:t0 + T],
                                        acc[("q", 0)])
                    continue
                # ---- RoPE on q, k (pair-swap matmul + 3 DVE ops) ----
                for qk in ("q", "k"):
                    for h in range(HLOC):
                        a = acc[(qk, h)]
                        for s4 in range(NTS):
                            sl = slice(s4 * TS, (s4 + 1) * TS)
                            psw_ps = ps.tile([128, TS], f32, tag="mm", bufs=3,
                                             name="ps_sw")
                            nc.tensor.matmul(psw_ps, lhsT=psw_sb, rhs=a[:, sl],
                                             start=True, stop=True)
                            t1 = tpool.tile([128, TS], bf16, tag="rt1",
                                            name="rope_t1")
                            nc.vector.tensor_mul(t1, a[:, sl], cos_sb[:, sl])
                            t2 = tpool.tile([128, TS], bf16, tag="rt2",
                                            name="rope_t2")
                            nc.vector.tensor_mul(t2, psw_ps, sin_sb[:, sl])
                            nc.vector.tensor_add(a[:, sl], t1, t2)

                if phases == "AR":
                    nc.gpsimd.dma_start(out_d[0:128, t0:t0 + T],
                                        acc[("q", 0)])
                    continue
                # ---- phase B: attention (2 j-tiles per exp group) ----
                o_sb = {}
                for h in range(HLOC):
                    O = opool.tile([128, T], bf16, tag="o", name=f"o_{b}{h}")
                    o_sb[h] = O
                    kacc, qacc = acc[("k", h)], acc[("q", h)]
                    for s in range(NTS):
                        isl = slice(s * TS, (s + 1) * TS)
                        o_ps = ps.tile([128, TS], f32, tag="mm", bufs=3,
                                       name="ps_o")
                        d_ps = (None if OPTS.get("no_ones") else
                                ps.tile([1, TS], f32, tag="den", bufs=1,
                                        name="ps_den"))
                        njt = 4 * (s + 1)
                        ngrp = njt // 2
                        pend = None

                        def flush_pend(pend):
                            g0, p0 = pend
                            for l in range(2):
                                jj = 2 * g0 + l
                                if not OPTS.get("no_ones"):
                                    nc.tensor.matmul(
                                        d_ps, lhsT=one_sb, rhs=p0[:, l, :],
                                        start=(jj == 0),
                                        stop=(jj == njt - 1))
                                nc.tensor.matmul(
                                    o_ps,
                                    lhsT=vtok[:, jj, h * 128:(h + 1) * 128],
                                    rhs=p0[:, l, :],
                                    start=(jj == 0), stop=(jj == njt - 1))

                        for g in range(ngrp):
                            s_ps = ps.tile([128, 2, TS], f32, tag="s2",
                                           bufs=2, name="ps_s")
                            for l in range(2):
                                jj = 2 * g + l
                                nc.tensor.matmul(
                                    s_ps[:, l, :],
                                    lhsT=kacc[:, jj * 128:(jj + 1) * 128],
                                    rhs=qacc[:, isl], start=True, stop=True)
                            p_sb = ppool.tile([128, 2, TS], bf16, tag="p",
                                              name="p_sb")
                            if OPTS.get("no_exp"):
                                nc.vector.tensor_copy(p_sb, s_ps)
                            else:
                                nc.scalar.activation(
                                    p_sb, s_ps,
                                    mybir.ActivationFunctionType.Exp,
                                    scale=SCALE)
                            if g >= 2 * s and not OPTS.get("no_mask"):
                                q0 = 2 * (g - 2 * s)
                                nc.vector.tensor_mul(
                                    p_sb, p_sb, msk_sb[:, q0:q0 + 2, :])
                            if pend is not None:
                                flush_pend(pend)
                            pend = (g, p_sb)
                        flush_pend(pend)
                        if OPTS.get("no_ones"):
                            nc.vector.tensor_copy(O[:, isl], o_ps)
                        else:
                            den_sb = dpool.tile([1, TS], f32, tag="den_sb",
                                                name="den_sb")
                            nc.scalar.copy(den_sb, d_ps)
                            den_bc = dpool.tile([128, TS], f32, tag="den_bc",
                                                name="den_bc")
                            nc.gpsimd.partition_broadcast(den_bc, den_sb)
                            den_rc = dpool.tile([128, TS], f32, tag="den_rc",
                                                name="den_rc")
                            nc.vector.reciprocal(den_rc, den_bc)
                            nc.vector.tensor_mul(O[:, isl], o_ps, den_rc)

                if phases == "AB":
                    for h in range(HLOC):
                        nc.gpsimd.dma_start(
                            out_d[h * 128:(h + 1) * 128, t0:t0 + T], o_sb[h])
                    continue
                # ---- phase C: output projection (partial) ----
                for u in range(C // 128):
                    for ts in range(NTS):
                        pj = ps.tile([128, TS], f32, tag="mm", bufs=3,
                                     name="ps_pj")
                        for h in range(HLOC):
                            nc.tensor.matmul(
                                pj, lhsT=wp_sb[:, h, u * 128:(u + 1) * 128],
                                rhs=o_sb[h][:, ts * TS:(ts + 1) * TS],
                                start=(h == 0), stop=(h == HLOC - 1))
                        ot = stpool.tile([128, TS], bf16, tag="ot", name="ot")
                        if (u * NTS + ts) % 2 == 0:
                            nc.vector.tensor_copy(ot, pj)
                        else:
                            nc.scalar.copy(ot, pj)
                        nc.sync.dma_start(
                            out_d[u * 128:(u + 1) * 128,
                                  t0 + ts * TS: t0 + (ts + 1) * TS], ot)

    nc.compile()
    return nc


def _host_prep(x, cos, sin, w_qkv, w_proj):
    bf = ml_dtypes.bfloat16
    x = np.asarray(x, dtype=np.float32)
    cos = np.asarray(cos, dtype=np.float32)
    sin = np.asarray(sin, dtype=np.float32)
    w_qkv = np.asarray(w_qkv, dtype=np.float32)
    w_proj = np.asarray(w_proj, dtype=np.float32)

    xt = np.ascontiguousarray(x.reshape(BT, C).T)          # [C, BT]
    cos2 = np.ascontiguousarray(np.repeat(cos.T, 2, axis=0)).astype(bf)
    sin2 = np.repeat(sin.T, 2, axis=0)
    sin2[0::2] *= -1.0
    sin2 = np.ascontiguousarray(sin2).astype(bf)
    psw = np.zeros((128, 128), np.float32)
    idx = np.arange(128)
    psw[idx, idx ^ 1] = 1.0
    psw = psw.astype(bf)
    idn = np.eye(128, dtype=np.float32).astype(bf)
    ones = np.ones((128, 1), np.float32).astype(bf)
    masks = np.zeros((4, 128, TS), np.float32)
    ii = np.arange(TS)[None, :]
    pj = np.arange(128)[:, None]
    for q in range(4):
        masks[q] = (ii >= pj + 128 * q).astype(np.float32)
    masks = masks.astype(bf)

    shared = {"xt": xt, "cos2": cos2, "sin2": sin2, "psw": psw,
              "idn": idn, "ones": ones, "masks": masks}
    in_maps = []
    for r in range(NCORES):
        f0 = FLOC * r
        m = dict(shared)
        m["wq"] = np.ascontiguousarray(w_qkv[f0:f0 + FLOC].T)
        m["wk"] = np.ascontiguousarray(w_qkv[C + f0:C + f0 + FLOC].T)
        m["wv"] = np.ascontiguousarray(w_qkv[2 * C + f0:2 * C + f0 + FLOC].T)
        m["wp"] = np.ascontiguousarray(w_proj[:, f0:f0 + FLOC].T).astype(bf)
        in_maps.append(m)
    return in_maps


def _run(in_maps, trace=False):
    from concourse.bass_utils import run_bass_kernel_spmd
    if "nc" not in _cache:
        _cache["nc"] = _build_nc()
    nc = _cache["nc"]
    res = run_bass_kernel_spmd(nc, in_maps, core_ids=list(range(NCORES)),
                               trace=trace)
    total = np.zeros((C, BT), np.float64)
    for r in range(NCORES):
        total += res.results[r]["out"].astype(np.float64)
    out = total.T.reshape(B, T, C).astype(np.float32)
    return out, res


def kernel(x, cos, sin, w_qkv, w_proj):
    in_maps = _host_prep(x, cos, sin, w_qkv, w_proj)
    out, _ = _run(in_maps, trace=False)
    return out


def kernel_traced(x, cos, sin, w_qkv, w_proj):
    """Like kernel() but also returns BassKernelResults with exec_time_ns."""
    in_maps = _host_prep(x, cos, sin, w_qkv, w_proj)
    return _run(in_maps, trace=True)



# revision 8
# speedup vs baseline: 1.2470x; 1.2470x over previous
"""Tensor-parallel causal self-attention on 8 TRN2 NeuronCores.

Sharding: head-parallel. Core r owns heads {2r, 2r+1} (256 of 2048 qkv
features). qkv weight column-sharded, proj weight row-sharded; each core
returns a partial projection output [2048, 4096] (feature-major), host sums
across cores and transposes back to (B, T, C).

On-core dataflow (all feature-major: features on partitions, tokens free):
  xT [2048c, 4096t] (replicated)
  qkv proj  : psum[f,t] += W[c,f]^T x[c,t]  (float32r, full PE rate)
  RoPE      : fused into the qkv phase per 512-token slice: pair-swap via a
              128x128 swap-matrix matmul + 3 DVE ops, overlapped with the
              next slice's projection matmuls
  scores    : S^T[j,i] = k^T[d,j]^T @ q^T[d,i]  (bf16)
  softmax   : P = exp(S*scale) (no max-sub; logits are O(1)), causal via
              multiplicative bf16 masks on diagonal blocks; denominator
              accumulated on DVE (bf16), then one all-ones [128,128] matmul
              does partition-sum + broadcast in a single PE op
  PV        : O^T[d,i] += v[j,d]^T @ P^T[j,i]  (bf16), divide by denom
  proj      : interleaved per 512-token slice right after both heads finish
              that slice; one batched DMA per (batch, slice) stores the
              partial projection
"""

import sys

if '/opt/trn_rl_repo' not in sys.path:
    sys.path.insert(0, '/opt/trn_rl_repo')

import numpy as np
import ml_dtypes

B, T, C = 2, 2048, 2048
H, HD = 16, 128
NCORES = 8
HLOC = H // NCORES          # 2 heads per core
FLOC = HLOC * HD            # 256 features per core
BT = B * T                  # 4096 tokens
CT = C // 128               # 16 contraction tiles
TS = 512                    # token slice
NTS = T // TS               # 4 t-slices per batch
NJT = T // 128              # 16 key tiles per batch
NU = C // 128               # 16 output row tiles
SCALE = 1.0 / float(np.sqrt(HD))

_cache = {}
OPTS = {}


def _build_nc(loop_n=None, phases="ABC"):
    """Build the SPMD kernel. loop_n wraps the whole computation in an
    on-device For_i loop (benchmarking only — amortizes dispatch overhead).
    phases: "A" qkv+rope only, "AB" +attention, "ABC" full."""
    import contextlib

    import concourse.bacc as bacc
    import concourse.mybir as mybir
    from concourse.tile import TileContext

    f32 = mybir.dt.float32
    f32r = mybir.dt.float32r
    bf16 = mybir.dt.bfloat16

    nc = bacc.Bacc("TRN2", target_bir_lowering=False, debug=False,
                   enable_partition_id=False)

    xt_d = nc.dram_tensor("xt", [C, BT], f32r, kind="ExternalInput")
    wq_d = nc.dram_tensor("wq", [C, FLOC], f32r, kind="ExternalInput")
    wk_d = nc.dram_tensor("wk", [C, FLOC], f32r, kind="ExternalInput")
    wv_d = nc.dram_tensor("wv", [C, FLOC], f32r, kind="ExternalInput")
    wp_d = nc.dram_tensor("wp", [FLOC, C], bf16, kind="ExternalInput")
    cos_d = nc.dram_tensor("cos2", [128, T], bf16, kind="ExternalInput")
    sin_d = nc.dram_tensor("sin2", [128, T], bf16, kind="ExternalInput")
    psw_d = nc.dram_tensor("psw", [128, 128], bf16, kind="ExternalInput")
    onm_d = nc.dram_tensor("onesmat", [128, 128], bf16, kind="ExternalInput")
    msk_d = nc.dram_tensor("masks", [4, 128, TS], bf16, kind="ExternalInput")
    out_d = nc.dram_tensor("out", [C, BT], bf16, kind="ExternalOutput")

    with TileContext(nc) as tc:
        with (
            tc.tile_pool(name="cpool", bufs=1) as cpool,
            tc.tile_pool(name="wpool", bufs=1) as wpool,
            tc.tile_pool(name="xpool", bufs=22) as xpool,
            tc.tile_pool(name="accpool", bufs=2) as accpool,
            tc.tile_pool(name="vpool", bufs=2) as vpool,
            tc.tile_pool(name="tpool", bufs=3) as tpool,
            tc.tile_pool(name="ppool", bufs=4) as ppool,
            tc.tile_pool(name="opool", bufs=4) as opool,
            tc.tile_pool(name="dapool", bufs=2) as dapool,
            tc.tile_pool(name="rcpool", bufs=2) as rcpool,
            tc.tile_pool(name="otpool", bufs=3) as otpool,
            tc.tile_pool(name="ps", bufs=8, space="PSUM") as ps,
        ):
            # ---- constants & weights (resident) ----
            cos_sb = cpool.tile([128, T], bf16, name="cos_sb")
            nc.sync.dma_start(cos_sb, cos_d[:, :])
            sin_sb = cpool.tile([128, T], bf16, name="sin_sb")
            nc.sync.dma_start(sin_sb, sin_d[:, :])
            psw_sb = cpool.tile([128, 128], bf16, name="psw_sb")
            nc.sync.dma_start(psw_sb, psw_d[:, :])
            onm_sb = cpool.tile([128, 128], bf16, name="onm_sb")
            nc.sync.dma_start(onm_sb, onm_d[:, :])
            msk_sb = cpool.tile([128, 4, TS], bf16, name="msk_sb")
            nc.sync.dma_start(msk_sb, msk_d.rearrange("q p i -> p q i"))

            w_sb = {}
            for nm, d in (("q", wq_d), ("k", wk_d), ("v", wv_d)):
                w = wpool.tile([128, CT, FLOC], f32r, name=f"w{nm}_sb")
                nc.sync.dma_start(w, d.rearrange("(ct p) f -> p ct f", p=128))
                w_sb[nm] = w
            wp_sb = wpool.tile([128, HLOC, C], bf16, name="wp_sb")
            nc.sync.dma_start(wp_sb, wp_d.rearrange("(hh p) u -> p hh u", p=128))

            loop_cm = (tc.For_i(0, loop_n, 1) if loop_n
                       else contextlib.nullcontext())
            with loop_cm:
              for b in range(B):
                t0 = b * T
                # ---- phase A: qkv projection (float32r) + fused RoPE ----
                acc = {}
                for qk in ("q", "k"):
                    for h in range(HLOC):
                        acc[(qk, h)] = accpool.tile(
                            [128, T], bf16, tag=f"acc_{qk}{h}",
                            name=f"acc_{qk}{h}_{b}")
                vtok = vpool.tile([128, NJT, FLOC], bf16, tag="vtok",
                                  name=f"vtok_{b}")

                # RoPE for a finished qkv slice is deferred by one chain so
                # the psum->sbuf copy hides under the next chain's matmuls
                # (PE executes in emission order).
                rope_pend = []

                def rope_emit(a, sl):
                    psw_ps = ps.tile([128, TS], f32, tag="mm", bufs=3,
                                     name="ps_sw")
                    nc.tensor.matmul(psw_ps, lhsT=psw_sb, rhs=a,
                                     start=True, stop=True)
                    t1 = tpool.tile([128, TS], bf16, tag="rt1",
                                    name="rope_t1")
                    nc.vector.tensor_mul(t1, a, cos_sb[:, sl])
                    t2 = tpool.tile([128, TS], bf16, tag="rt2",
                                    name="rope_t2")
                    nc.vector.tensor_mul(t2, psw_ps, sin_sb[:, sl])
                    nc.vector.tensor_add(a, t1, t2)

                for ts in range(NTS):
                    sl = slice(ts * TS, (ts + 1) * TS)
                    xts = []
                    for c in range(CT):
                        xt_t = xpool.tile([128, TS], f32r, tag="xt_t",
                                          name="xt_t")
                        nc.sync.dma_start(
                            xt_t,
                            xt_d[c * 128:(c + 1) * 128,
                                 t0 + ts * TS: t0 + (ts + 1) * TS])
                        xts.append(xt_t)
                    for nm in ("q", "k"):
                        for h in range(HLOC):
                            pq = ps.tile([128, TS], f32, tag="mm", bufs=3,
                                         name="ps_qkv")
                            for c in range(CT):
                                nc.tensor.matmul(
                                    pq,
                                    lhsT=w_sb[nm][:, c, h * 128:(h + 1) * 128],
                                    rhs=xts[c],
                                    start=(c == 0), stop=(c == CT - 1))
                            a = acc[(nm, h)][:, sl]
                            nc.scalar.copy(a, pq)
                            if rope_pend:
                                rope_emit(*rope_pend.pop())
                            rope_pend.append((a, sl))
                    # v directly in token-major: out[t, f] with xT stationary
                    for tt in range(4):
                        pv = ps.tile([128, FLOC], f32, tag="mm", bufs=3,
                                     name="ps_v")
                        for c in range(CT):
                            nc.tensor.matmul(
                                pv,
                                lhsT=xts[c][:, tt * 128:(tt + 1) * 128],
                                rhs=w_sb["v"][:, c, :],
                                start=(c == 0), stop=(c == CT - 1))
                        nc.scalar.copy(vtok[:, ts * 4 + tt, :], pv)
                        if tt == 1 and rope_pend:
                            rope_emit(*rope_pend.pop())
                if rope_pend:
                    rope_emit(*rope_pend.pop())

                if phases == "A":
                    nc.gpsimd.dma_start(out_d[0:128, t0:t0 + T],
                                        acc[("q", 0)])
                    continue
                # ---- phase B + C interleaved per 512-token slice ----
                for s in range(NTS):
                    isl = slice(s * TS, (s + 1) * TS)
                    o_sl = {}
                    for h in range(HLOC):
                        kacc, qacc = acc[("k", h)], acc[("q", h)]
                        o_ps = ps.tile([128, TS], f32, tag="mm", bufs=3,
                                       name="ps_o")
                        den = dapool.tile([128, 2, TS], bf16, tag="den",
                                          name="den_acc")
                        njt = 4 * (s + 1)
                        ngrp = njt // 2
                        pend = None

                        def flush_pend(pend):
                            g0, p0 = pend
                            for l in range(2):
                                jj = 2 * g0 + l
                                nc.tensor.matmul(
                                    o_ps,
                                    lhsT=vtok[:, jj, h * 128:(h + 1) * 128],
                                    rhs=p0[:, l, :],
                                    start=(jj == 0), stop=(jj == njt - 1))

                        for g in range(ngrp):
                            s_ps = ps.tile([128, 2, TS], f32, tag="s2",
                                           bufs=2, name="ps_s")
                            for l in range(2):
                                jj = 2 * g + l
                                nc.tensor.matmul(
                                    s_ps[:, l, :],
                                    lhsT=kacc[:, jj * 128:(jj + 1) * 128],
                                    rhs=qacc[:, isl], start=True, stop=True)
                            p_sb = ppool.tile([128, 2, TS], bf16, tag="p",
                                              name="p_sb")
                            if OPTS.get("no_exp"):
                                nc.vector.tensor_copy(p_sb, s_ps)
                            else:
                                nc.scalar.activation(
                                    p_sb, s_ps,
                                    mybir.ActivationFunctionType.Exp,
                                    scale=SCALE)
                            if g >= 2 * s and not OPTS.get("no_mask"):
                                q0 = 2 * (g - 2 * s)
                                nc.vector.tensor_mul(
                                    p_sb, p_sb, msk_sb[:, q0:q0 + 2, :])
                            if g == 0:
                                nc.vector.tensor_copy(den, p_sb)
                            else:
                                nc.vector.tensor_add(den, den, p_sb)
                            if pend is not None:
                                flush_pend(pend)
                            pend = (g, p_sb)
                        flush_pend(pend)
                        # denominator: partition-sum + broadcast in one
                        # all-ones matmul, then reciprocal-multiply
                        db_ps = ps.tile([128, TS], f32, tag="db", bufs=1,
                                        name="ps_db")
                        nc.tensor.matmul(db_ps, lhsT=onm_sb, rhs=den[:, 0, :],
                                         start=True, stop=False)
                        nc.tensor.matmul(db_ps, lhsT=onm_sb, rhs=den[:, 1, :],
                                         start=False, stop=True)
                        rc = rcpool.tile([128, TS], f32, tag="rc", name="rc")
                        nc.vector.reciprocal(rc, db_ps)
                        O = opool.tile([128, TS], bf16, tag="o",
                                       name=f"o_{b}{h}{s}")
                        nc.vector.tensor_mul(O, o_ps, rc)
                        o_sl[h] = O

                    if phases == "AB":
                        for h in range(HLOC):
                            nc.gpsimd.dma_start(
                                out_d[h * 128:(h + 1) * 128,
                                      t0 + s * TS:t0 + (s + 1) * TS],
                                o_sl[h])
                        continue
                    # ---- phase C for this slice (4 u-tiles per DMA) ----
                    for u0 in range(0, NU, 4):
                        ot = otpool.tile([128, 4, TS], bf16, tag="ot",
                                         name="ot")
                        for du in range(4):
                            u = u0 + du
                            pj = ps.tile([128, TS], f32, tag="mm", bufs=3,
                                         name="ps_pj")
                            for h in range(HLOC):
                                nc.tensor.matmul(
                                    pj,
                                    lhsT=wp_sb[:, h, u * 128:(u + 1) * 128],
                                    rhs=o_sl[h],
                                    start=(h == 0), stop=(h == HLOC - 1))
                            if u % 2 == 0:
                                nc.vector.tensor_copy(ot[:, du, :], pj)
                            else:
                                nc.scalar.copy(ot[:, du, :], pj)
                        nc.sync.dma_start(
                            out_d.rearrange("(u p) t -> p u t", p=128)[
                                :, u0:u0 + 4,
                                t0 + s * TS: t0 + (s + 1) * TS],
                            ot)

    nc.compile()
    return nc


def _host_prep(x, cos, sin, w_qkv, w_proj):
    bf = ml_dtypes.bfloat16
    x = np.asarray(x, dtype=np.float32)
    cos = np.asarray(cos, dtype=np.float32)
    sin = np.asarray(sin, dtype=np.float32)
    w_qkv = np.asarray(w_qkv, dtype=np.float32)
    w_proj = np.asarray(w_proj, dtype=np.float32)

    xt = np.ascontiguousarray(x.reshape(BT, C).T)          # [C, BT]
    cos2 = np.ascontiguousarray(np.repeat(cos.T, 2, axis=0)).astype(bf)
    sin2 = np.repeat(sin.T, 2, axis=0)
    sin2[0::2] *= -1.0
    sin2 = np.ascontiguousarray(sin2).astype(bf)
    psw = np.zeros((128, 128), np.float32)
    idx = np.arange(128)
    psw[idx, idx ^ 1] = 1.0
    psw = psw.astype(bf)
    onesmat = np.ones((128, 128), np.float32).astype(bf)
    masks = np.zeros((4, 128, TS), np.float32)
    ii = np.arange(TS)[None, :]
    pj = np.arange(128)[:, None]
    for q in range(4):
        masks[q] = (ii >= pj + 128 * q).astype(np.float32)
    masks = masks.astype(bf)

    shared = {"xt": xt, "cos2": cos2, "sin2": sin2, "psw": psw,
              "onesmat": onesmat, "masks": masks}
    in_maps = []
    for r in range(NCORES):
        f0 = FLOC * r
        m = dict(shared)
        m["wq"] = np.ascontiguousarray(w_qkv[f0:f0 + FLOC].T)
        m["wk"] = np.ascontiguousarray(w_qkv[C + f0:C + f0 + FLOC].T)
        m["wv"] = np.ascontiguousarray(w_qkv[2 * C + f0:2 * C + f0 + FLOC].T)
        m["wp"] = np.ascontiguousarray(w_proj[:, f0:f0 + FLOC].T).astype(bf)
        in_maps.append(m)
    return in_maps


def _run(in_maps, trace=False):
    from concourse.bass_utils import run_bass_kernel_spmd
    if "nc" not in _cache:
        _cache["nc"] = _build_nc()
    nc = _cache["nc"]
    res = run_bass_kernel_spmd(nc, in_maps, core_ids=list(range(NCORES)),
                               trace=trace)
    total = np.zeros((C, BT), np.float64)
    for r in range(NCORES):
        total += res.results[r]["out"].astype(np.float64)
    out = total.T.reshape(B, T, C).astype(np.float32)
    return out, res


def kernel(x, cos, sin, w_qkv, w_proj):
    in_maps = _host_prep(x, cos, sin, w_qkv, w_proj)
    out, _ = _run(in_maps, trace=False)
    return out


def kernel_traced(x, cos, sin, w_qkv, w_proj):
    """Like kernel() but also returns BassKernelResults with exec_time_ns."""
    in_maps = _host_prep(x, cos, sin, w_qkv, w_proj)
    return _run(in_maps, trace=True)


# revision 11
# speedup vs baseline: 1.3134x; 1.0533x over previous
"""Tensor-parallel causal self-attention on 8 TRN2 NeuronCores.

Sharding: head-parallel. Core r owns heads {2r, 2r+1} (256 of 2048 qkv
features). qkv weight column-sharded, proj weight row-sharded; each core
returns a partial projection output [2048, 4096] (feature-major), host sums
across cores and transposes back to (B, T, C).

On-core dataflow (all feature-major: features on partitions, tokens free):
  xT [2048c, 4096t] (replicated)
  qkv proj  : psum[f,t] += W[c,f]^T x[c,t]  (bf16)
  RoPE      : fused into the qkv phase per 512-token slice: pair-swap via a
              128x128 swap-matrix matmul + 3 DVE ops, overlapped with the
              next slice's projection matmuls
  scores    : S^T[j,i] = k^T[d,j]^T @ q^T[d,i]  (bf16)
  softmax   : P = exp(S*scale) (no max-sub; logits are O(1)), causal via
              multiplicative bf16 masks on diagonal blocks; denominator
              accumulated on DVE (bf16), then one all-ones [128,128] matmul
              does partition-sum + broadcast in a single PE op
  PV        : O^T[d,i] += v[j,d]^T @ P^T[j,i]  (bf16), divide by denom
  proj      : interleaved per 512-token slice right after both heads finish
              that slice; one batched DMA per (batch, slice) stores the
              partial projection
"""

import sys

if '/opt/trn_rl_repo' not in sys.path:
    sys.path.insert(0, '/opt/trn_rl_repo')

import numpy as np
import ml_dtypes

B, T, C = 2, 2048, 2048
H, HD = 16, 128
NCORES = 8
HLOC = H // NCORES          # 2 heads per core
FLOC = HLOC * HD            # 256 features per core
BT = B * T                  # 4096 tokens
CT = C // 128               # 16 contraction tiles
TS = 512                    # token slice
NTS = T // TS               # 4 t-slices per batch
NJT = T // 128              # 16 key tiles per batch
NU = C // 128               # 16 output row tiles
SCALE = 1.0 / float(np.sqrt(HD))

_cache = {}
OPTS = {}


def _build_nc(loop_n=None, phases="ABC"):
    """Build the SPMD kernel. loop_n wraps the whole computation in an
    on-device For_i loop (benchmarking only — amortizes dispatch overhead).
    phases: "A" qkv+rope only, "AB" +attention, "ABC" full."""
    import contextlib

    import concourse.bacc as bacc
    import concourse.mybir as mybir
    from concourse.tile import TileContext

    f32 = mybir.dt.float32
    f32r = mybir.dt.float32r
    bf16 = mybir.dt.bfloat16

    nc = bacc.Bacc("TRN2", target_bir_lowering=False, debug=False,
                   enable_partition_id=False)

    xt_d = nc.dram_tensor("xt", [C, BT], bf16, kind="ExternalInput")
    wq_d = nc.dram_tensor("wq", [C, FLOC], bf16, kind="ExternalInput")
    wk_d = nc.dram_tensor("wk", [C, FLOC], bf16, kind="ExternalInput")
    wv_d = nc.dram_tensor("wv", [C, FLOC], bf16, kind="ExternalInput")
    wp_d = nc.dram_tensor("wp", [FLOC, C], bf16, kind="ExternalInput")
    cos_d = nc.dram_tensor("cos2", [128, T], bf16, kind="ExternalInput")
    sin_d = nc.dram_tensor("sin2", [128, T], bf16, kind="ExternalInput")
    psw_d = nc.dram_tensor("psw", [128, 128], bf16, kind="ExternalInput")
    onm_d = nc.dram_tensor("onesmat", [128, 128], bf16, kind="ExternalInput")
    msk_d = nc.dram_tensor("masks", [4, 128, TS], bf16, kind="ExternalInput")
    out_d = nc.dram_tensor("out", [C, BT], bf16, kind="ExternalOutput")

    with TileContext(nc) as tc:
        with (
            tc.tile_pool(name="cpool", bufs=1) as cpool,
            tc.tile_pool(name="wpool", bufs=1) as wpool,
            tc.tile_pool(name="xpool", bufs=36) as xpool,
            tc.tile_pool(name="accpool", bufs=2) as accpool,
            tc.tile_pool(name="vpool", bufs=2) as vpool,
            tc.tile_pool(name="tpool", bufs=3) as tpool,
            tc.tile_pool(name="ppool", bufs=6) as ppool,
            tc.tile_pool(name="opool", bufs=4) as opool,
            tc.tile_pool(name="dapool", bufs=2) as dapool,
            tc.tile_pool(name="rcpool", bufs=2) as rcpool,
            tc.tile_pool(name="otpool", bufs=3) as otpool,
            tc.tile_pool(name="ps", bufs=8, space="PSUM") as ps,
        ):
            # ---- constants & weights (resident) ----
            cos_sb = cpool.tile([128, T], bf16, name="cos_sb")
            nc.sync.dma_start(cos_sb, cos_d[:, :])
            sin_sb = cpool.tile([128, T], bf16, name="sin_sb")
            nc.sync.dma_start(sin_sb, sin_d[:, :])
            psw_sb = cpool.tile([128, 128], bf16, name="psw_sb")
            nc.sync.dma_start(psw_sb, psw_d[:, :])
            onm_sb = cpool.tile([128, 128], bf16, name="onm_sb")
            nc.sync.dma_start(onm_sb, onm_d[:, :])
            msk_sb = cpool.tile([128, 4, TS], bf16, name="msk_sb")
            nc.sync.dma_start(msk_sb, msk_d.rearrange("q p i -> p q i"))

            w_sb = {}
            for nm, d in (("q", wq_d), ("k", wk_d), ("v", wv_d)):
                w = wpool.tile([128, CT, FLOC], bf16, name=f"w{nm}_sb")
                nc.sync.dma_start(w, d.rearrange("(ct p) f -> p ct f", p=128))
                w_sb[nm] = w
            wp_sb = wpool.tile([128, HLOC, C], bf16, name="wp_sb")
            nc.sync.dma_start(wp_sb, wp_d.rearrange("(hh p) u -> p hh u", p=128))

            loop_cm = (tc.For_i(0, loop_n, 1) if loop_n
                       else contextlib.nullcontext())
            with loop_cm:
              for b in range(B):
                t0 = b * T
                # ---- phase A: qkv projection (float32r) + fused RoPE ----
                acc = {}
                for qk in ("q", "k"):
                    for h in range(HLOC):
                        acc[(qk, h)] = accpool.tile(
                            [128, T], bf16, tag=f"acc_{qk}{h}",
                            name=f"acc_{qk}{h}_{b}")
                vtok = vpool.tile([128, NJT, FLOC], bf16, tag="vtok",
                                  name=f"vtok_{b}")

                # RoPE for a finished qkv slice is deferred by one chain so
                # the psum->sbuf copy hides under the next chain's matmuls
                # (PE executes in emission order).
                rope_pend = []

                def rope_emit(a, sl):
                    psw_ps = ps.tile([128, TS], f32, tag="mm", bufs=3,
                                     name="ps_sw")
                    nc.tensor.matmul(psw_ps, lhsT=psw_sb, rhs=a,
                                     start=True, stop=True)
                    t1 = tpool.tile([128, TS], bf16, tag="rt1",
                                    name="rope_t1")
                    nc.vector.tensor_mul(t1, a, cos_sb[:, sl])
                    t2 = tpool.tile([128, TS], bf16, tag="rt2",
                                    name="rope_t2")
                    nc.vector.tensor_mul(t2, psw_ps, sin_sb[:, sl])
                    nc.vector.tensor_add(a, t1, t2)

                for ts in range(NTS):
                    sl = slice(ts * TS, (ts + 1) * TS)
                    xts = []
                    for c in range(CT):
                        xt_t = xpool.tile([128, TS], bf16, tag="xt_t",
                                          name="xt_t")
                        nc.sync.dma_start(
                            xt_t,
                            xt_d[c * 128:(c + 1) * 128,
                                 t0 + ts * TS: t0 + (ts + 1) * TS])
                        xts.append(xt_t)
                    for nm in ("q", "k"):
                        for h in range(HLOC):
                            pq = ps.tile([128, TS], f32, tag="mm", bufs=3,
                                         name="ps_qkv")
                            for c in range(CT):
                                nc.tensor.matmul(
                                    pq,
                                    lhsT=w_sb[nm][:, c, h * 128:(h + 1) * 128],
                                    rhs=xts[c],
                                    start=(c == 0), stop=(c == CT - 1))
                            a = acc[(nm, h)][:, sl]
                            nc.scalar.copy(a, pq)
                            if rope_pend:
                                rope_emit(*rope_pend.pop())
                            rope_pend.append((a, sl))
                    # v directly in token-major: out[t, f] with xT stationary
                    for tt in range(4):
                        pv = ps.tile([128, FLOC], f32, tag="mm", bufs=3,
                                     name="ps_v")
                        for c in range(CT):
                            nc.tensor.matmul(
                                pv,
                                lhsT=xts[c][:, tt * 128:(tt + 1) * 128],
                                rhs=w_sb["v"][:, c, :],
                                start=(c == 0), stop=(c == CT - 1))
                        nc.scalar.copy(vtok[:, ts * 4 + tt, :], pv)
                        if tt == 1 and rope_pend:
                            rope_emit(*rope_pend.pop())
                if rope_pend:
                    rope_emit(*rope_pend.pop())

                if phases == "A":
                    nc.gpsimd.dma_start(out_d[0:128, t0:t0 + T],
                                        acc[("q", 0)])
                    continue
                # ---- phase B + C interleaved per 512-token slice ----
                for s in range(NTS):
                    isl = slice(s * TS, (s + 1) * TS)
                    o_sl = {}
                    for h in range(HLOC):
                        kacc, qacc = acc[("k", h)], acc[("q", h)]
                        o_ps = ps.tile([128, TS], f32, tag="mm", bufs=3,
                                       name="ps_o")
                        den = dapool.tile([128, TS], bf16, tag="den",
                                          name="den_acc")
                        njt = 4 * (s + 1)
                        # PV matmuls deferred one block so exp overlaps PE
                        pend = []

                        def flush_pend():
                            for jj, p_ap, c0 in pend:
                                nc.tensor.matmul(
                                    o_ps[:, c0:],
                                    lhsT=vtok[:, jj, h * 128:(h + 1) * 128],
                                    rhs=p_ap,
                                    start=(jj == 0), stop=(jj == njt - 1))
                            pend.clear()

                        def den_acc_in(p_ap, c0, first):
                            if first:
                                nc.vector.tensor_copy(den, p_ap)
                            else:
                                nc.vector.tensor_add(den[:, c0:],
                                                     den[:, c0:], p_ap)

                        # off-diagonal j-tile pairs (full 512-col width)
                        for g in range(2 * s):
                            s_ps = ps.tile([128, 2, TS], f32, tag="s2",
                                           bufs=2, name="ps_s")
                            for l in range(2):
                                jj = 2 * g + l
                                nc.tensor.matmul(
                                    s_ps[:, l, :],
                                    lhsT=kacc[:, jj * 128:(jj + 1) * 128],
                                    rhs=qacc[:, isl], start=True, stop=True)
                            p_sb = ppool.tile([128, 2, TS], bf16, tag="p",
                                              name="p_sb")
                            nc.scalar.activation(
                                p_sb, s_ps,
                                mybir.ActivationFunctionType.Exp,
                                scale=SCALE)
                            den_acc_in(p_sb[:, 0, :], 0, g == 0)
                            nc.vector.tensor_add(den, den, p_sb[:, 1, :])
                            flush_pend()
                            pend.append((2 * g, p_sb[:, 0, :], 0))
                            pend.append((2 * g + 1, p_sb[:, 1, :], 0))
                        # diagonal block: 4 j-tiles at shrinking width
                        for d in range(4):
                            jj = 4 * s + d
                            c0 = 128 * d
                            w = TS - c0
                            s_ps = ps.tile([128, 2, TS], f32, tag="s2",
                                           bufs=2, name="ps_sd")
                            nc.tensor.matmul(
                                s_ps[:, 0, :w],
                                lhsT=kacc[:, jj * 128:(jj + 1) * 128],
                                rhs=qacc[:, s * TS + c0:(s + 1) * TS],
                                start=True, stop=True)
                            p_sb = ppool.tile([128, 2, TS], bf16, tag="p",
                                              name="p_sbd")
                            nc.scalar.activation(
                                p_sb[:, 0, :w], s_ps[:, 0, :w],
                                mybir.ActivationFunctionType.Exp,
                                scale=SCALE)
                            nc.vector.tensor_mul(
                                p_sb[:, 0, :w], p_sb[:, 0, :w],
                                msk_sb[:, d, c0:])
                            den_acc_in(p_sb[:, 0, :w], c0, jj == 0)
                            flush_pend()
                            pend.append((jj, p_sb[:, 0, :w], c0))
                        flush_pend()
                        # denominator: partition-sum + broadcast in one
                        # all-ones matmul, then reciprocal-multiply
                        db_ps = ps.tile([128, TS], f32, tag="db", bufs=1,
                                        name="ps_db")
                        nc.tensor.matmul(db_ps, lhsT=onm_sb, rhs=den,
                                         start=True, stop=True)
                        rc = rcpool.tile([128, TS], f32, tag="rc", name="rc")
                        nc.vector.reciprocal(rc, db_ps)
                        O = opool.tile([128, TS], bf16, tag="o",
                                       name=f"o_{b}{h}{s}")
                        nc.vector.tensor_mul(O, o_ps, rc)
                        o_sl[h] = O

                    if phases == "AB":
                        for h in range(HLOC):
                            nc.gpsimd.dma_start(
                                out_d[h * 128:(h + 1) * 128,
                                      t0 + s * TS:t0 + (s + 1) * TS],
                                o_sl[h])
                        continue
                    # ---- phase C for this slice (4 u-tiles per DMA) ----
                    for u0 in range(0, NU, 4):
                        ot = otpool.tile([128, 4, TS], bf16, tag="ot",
                                         name="ot")
                        for du in range(4):
                            u = u0 + du
                            pj = ps.tile([128, TS], f32, tag="mm", bufs=3,
                                         name="ps_pj")
                            for h in range(HLOC):
                                nc.tensor.matmul(
                                    pj,
                                    lhsT=wp_sb[:, h, u * 128:(u + 1) * 128],
                                    rhs=o_sl[h],
                                    start=(h == 0), stop=(h == HLOC - 1))
                            if u % 2 == 0:
                                nc.vector.tensor_copy(ot[:, du, :], pj)
                            else:
                                nc.scalar.copy(ot[:, du, :], pj)
                        nc.sync.dma_start(
                            out_d.rearrange("(u p) t -> p u t", p=128)[
                                :, u0:u0 + 4,
                                t0 + s * TS: t0 + (s + 1) * TS],
                            ot)

    nc.compile()
    return nc


def _host_prep(x, cos, sin, w_qkv, w_proj):
    bf = ml_dtypes.bfloat16
    x = np.asarray(x, dtype=np.float32)
    cos = np.asarray(cos, dtype=np.float32)
    sin = np.asarray(sin, dtype=np.float32)
    w_qkv = np.asarray(w_qkv, dtype=np.float32)
    w_proj = np.asarray(w_proj, dtype=np.float32)

    xt = np.ascontiguousarray(x.reshape(BT, C).T).astype(bf)  # [C, BT]
    cos2 = np.ascontiguousarray(np.repeat(cos.T, 2, axis=0)).astype(bf)
    sin2 = np.repeat(sin.T, 2, axis=0)
    sin2[0::2] *= -1.0
    sin2 = np.ascontiguousarray(sin2).astype(bf)
    psw = np.zeros((128, 128), np.float32)
    idx = np.arange(128)
    psw[idx, idx ^ 1] = 1.0
    psw = psw.astype(bf)
    onesmat = np.ones((128, 128), np.float32).astype(bf)
    masks = np.zeros((4, 128, TS), np.float32)
    ii = np.arange(TS)[None, :]
    pj = np.arange(128)[:, None]
    for q in range(4):
        masks[q] = (ii >= pj + 128 * q).astype(np.float32)
    masks = masks.astype(bf)

    shared = {"xt": xt, "cos2": cos2, "sin2": sin2, "psw": psw,
              "onesmat": onesmat, "masks": masks}
    in_maps = []
    for r in range(NCORES):
        f0 = FLOC * r
        m = dict(shared)
        m["wq"] = np.ascontiguousarray(w_qkv[f0:f0 + FLOC].T).astype(bf)
        m["wk"] = np.ascontiguousarray(w_qkv[C + f0:C + f0 + FLOC].T).astype(bf)
        m["wv"] = np.ascontiguousarray(w_qkv[2 * C + f0:2 * C + f0 + FLOC].T).astype(bf)
        m["wp"] = np.ascontiguousarray(w_proj[:, f0:f0 + FLOC].T).astype(bf)
        in_maps.append(m)
    return in_maps


def _run(in_maps, trace=False):
    from concourse.bass_utils import run_bass_kernel_spmd
    if "nc" not in _cache:
        _cache["nc"] = _build_nc()
    nc = _cache["nc"]
    res = run_bass_kernel_spmd(nc, in_maps, core_ids=list(range(NCORES)),
                               trace=trace)
    total = np.zeros((C, BT), np.float64)
    for r in range(NCORES):
        total += res.results[r]["out"].astype(np.float64)
    out = total.T.reshape(B, T, C).astype(np.float32)
    return out, res


def kernel(x, cos, sin, w_qkv, w_proj):
    in_maps = _host_prep(x, cos, sin, w_qkv, w_proj)
    out, _ = _run(in_maps, trace=False)
    return out


def kernel_traced(x, cos, sin, w_qkv, w_proj):
    """Like kernel() but also returns BassKernelResults with exec_time_ns."""
    in_maps = _host_prep(x, cos, sin, w_qkv, w_proj)
    return _run(in_maps, trace=True)
